# revision 34
# baseline (speedup 1.0000x reference)
"""Distributed Trainium2 Bass kernel for nn_AttentionLayer_25993142075512.

Sharding: 8 cores = 2 batches x 4 head-groups (4 heads each). Each core
computes its batch's q/k/v projections for its 4 heads, causal attention,
and a partial output projection o @ Wo[head_rows]. Host sums the 4
partials per batch and adds bo (plus the folded v-bias term).

v2.1 design notes:
  - qT[pr]/kT[pr] hold a HEAD PAIR: head 2pr at partitions 0:64, head 2pr+1
    at 64:128 (natural dim order). Score matmuls for the two heads go to PE
    row tiles (0,0)/(64,0) and run CONCURRENTLY (2x row tiling).
  - RoPE via rotate_every_two copies computed on-device: q2 = R^T q where R
    is a host-built +/-1 permutation [128,64] applied to qT/kT directly (one
    N=512 matmul per pair per sub instead of a full 8-chunk projection);
    rope = q*cos + q2*sin: 3 DVE ops per head per chunk, bases 0 mod 32.
  - vT produced directly in [keys, dims] layout (stationary = s_kv^T chunk,
    moving = Wv columns) - no transpose phase. v bias folded into bo on host.
  - oT matmuls full-K (M=65); softmax denominator comes free via a ones
    column appended to the transposed-v stationary.
  - exp for both heads fused in one ACT op [128, 2, w] over two psum banks;
    the causal diagonal mask is applied AFTER exp as a multiplicative
    0/1-triangle on the (otherwise idle) gpsimd engine, keeping the
    DVE queue out of the exp->oT critical chain.
  - Denominators summed into a free-dim-indexed [1,16,512] tile (engine
    partition bases must be 0 mod 32), spread by SBUF->SBUF DMA, one
    reciprocal, broadcast via one-hot E matmuls (64-mode), DVE multiply.
  - Unified schedule: per projection sub, both head pairs' attention
    groups run back to back; each group's normalize + out-projection is
    emitted one group later, overlapping the exp-paced stream. PSUM:
    proj/bc/po 2x2KB + sT 2x4KB + oT 2x2KB = 16KB exactly.
  - Consolidated host-packed weight DMAs; skvt input DMAs issued from the
    (otherwise idle) gpsimd queue to halve Sync descriptor-issue time.

Assumes mask_q == 1 (spec fill=ones); mask_kv handled exactly via exp bias.
"""

import sys, os, types, ctypes, contextlib

sys.path.insert(0, "/opt/trn_rl_repo")

import numpy as np
import ml_dtypes


def _install_axon_hooks():
    so = "/opt/axon/libaxon_pjrt.so"

    def _hook_factory(so_path):
        if not os.path.exists(so_path):
            return None
        lib = ctypes.CDLL(so_path)
        if not hasattr(lib, "axon_start_nrt_profile"):
            return None
        lib.axon_start_nrt_profile.argtypes = [
            ctypes.POINTER(ctypes.c_int64),
            ctypes.c_size_t,
        ]
        lib.axon_start_nrt_profile.restype = ctypes.c_int64
        lib.axon_stop_nrt_profile.argtypes = [ctypes.c_char_p]
        lib.axon_stop_nrt_profile.restype = ctypes.c_int64

        @contextlib.contextmanager
        def _hook(output_dir, device_ids):
            import jax

            jax.devices()
            if device_ids:
                ids = (ctypes.c_int64 * len(device_ids))(*device_ids)
                rc = lib.axon_start_nrt_profile(ids, len(device_ids))
            else:
                rc = lib.axon_start_nrt_profile(None, 0)
            if rc != 0:
                raise RuntimeError(f"axon_start_nrt_profile rc={rc}")
            try:
                yield
            finally:
                n = lib.axon_stop_nrt_profile(str(output_dir).encode())
                if n < 0:
                    raise RuntimeError(f"axon_stop_nrt_profile rc={n}")

        return _hook

    try:
        import antenv

        if "antenv.axon_hooks" not in sys.modules:
            hook = _hook_factory(so)
            mod = types.ModuleType("antenv.axon_hooks")
            mod.get_axon_ntff_profile_hook = lambda: hook
            mod.set_axon_ntff_profile_hook = lambda h: None
            antenv.axon_hooks = mod
            sys.modules["antenv.axon_hooks"] = mod
    except ImportError:
        pass
    from concourse import bass_utils

    bass_utils.upload_artifacts = lambda tmpdir: tmpdir


_install_axon_hooks()

from concourse import bass, bacc, tile, mybir  # noqa: E402

BF16 = mybir.dt.bfloat16
F32 = mybir.dt.float32
NPBF16 = ml_dtypes.bfloat16

B, N, DQ, DKV, H, DH, DOUT = 2, 2048, 1024, 1024, 16, 64, 1024
ROT = DH // 2  # 32
INF = 1.0e6
HPC = 4  # heads per core (2 pairs)
NB = N // 128  # 16 k-blocks
NG = NB // 4  # 4 q-block groups (512 cols each)
NS = 4  # projection subs (512 cols each)
VS = 66  # vg per-(kb,pr,hh) stride (64 v cols + ones col + pad)


def build_nc():
    nc = bacc.Bacc(None, target_bir_lowering=False)

    # inputs host-packed wave-major: [partition, chunk, wave, 512] so one
    # dma_start moves a whole 1MB column-wave (descriptor issue is ~0.6us per
    # dma_start on the issuing engine queue -- consolidation is critical)
    sqt_d = nc.declare_dram_parameter("sqt", [128, 8, N], BF16, isOutput=False)
    skvt_d = nc.declare_dram_parameter("skvt", [128, 8, N], BF16, isOutput=False)
    # pkq: wq0|wq1|rmat packed bf16 (q-side, needed first); pkkv: wk0|wk1|wv
    pkq_d = nc.declare_dram_parameter("pkq", [128, 2176], BF16, isOutput=False)
    pkkv_d = nc.declare_dram_parameter("pkkv", [128, 4096], BF16, isOutput=False)
    # pk32: bq(2)|bk(2)|bmask(16) packed as one f32 [128, 20] tensor
    pk32_d = nc.declare_dram_parameter("pk32", [128, 20], F32, isOutput=False)
    wo_d = nc.declare_dram_parameter("wo", [2, 128, DOUT], BF16, isOutput=False)
    cost_d = nc.declare_dram_parameter("cost", [32, N], BF16, isOutput=False)
    sint_d = nc.declare_dram_parameter("sint", [32, N], BF16, isOutput=False)
    mtile_d = nc.declare_dram_parameter("mtile", [128, 2, 128], BF16, isOutput=False)
    e64_d = nc.declare_dram_parameter("e64", [64, 1024], BF16, isOutput=False)
    out_ext = nc.declare_dram_parameter("out", [N, DOUT], BF16, isOutput=True)

    OQ = (0, 1024)      # wq pair offsets in pkq
    ORM = 2048          # rmat offset in pkq
    OK = (0, 1024)      # wk pair offsets in pkkv
    OWV = 2048          # wv offset in pkkv [128, 8*256]

    AF = mybir.ActivationFunctionType
    ALU = mybir.AluOpType

    with tile.TileContext(nc) as tc:
        with (
            tc.tile_pool(name="const", bufs=1) as cpool,
            tc.tile_pool(name="big", bufs=1) as bigpool,
            tc.tile_pool(name="small", bufs=8) as smallpool,
            tc.tile_pool(name="ptile", bufs=4) as ppool,
            tc.tile_pool(name="outsb", bufs=4) as outsb_pool,
        ):
            # ---- SBUF constants ----
            pkq = cpool.tile([128, 2176], BF16, tag="pkq", name="pkq")
            pkkv = cpool.tile([128, 4096], BF16, tag="pkkv", name="pkkv")
            pk32 = cpool.tile([128, 20], F32, tag="pk32", name="pk32")
            wo_sb = [cpool.tile([128, DOUT], BF16, tag=f"wo{p}", name=f"wo{p}") for p in range(2)]
            cost = cpool.tile([128, N], BF16, tag="cost", name="cost")
            sint = cpool.tile([128, N], BF16, tag="sint", name="sint")
            mtile = cpool.tile([128, 2, 128], BF16, tag="mtile", name="mtile")
            e64 = cpool.tile([64, 8, 128], BF16, tag="e64", name="e64")

            # full-resident transposed inputs as single tiles [128, chunk, N]
            sqt_sb = bigpool.tile([128, 8, N], BF16, tag="sqt", name="sqt")
            skvt_sb = bigpool.tile([128, 8, N], BF16, tag="skvt", name="skvt")

            # Stage-0 DMAs all run concurrently (pkq, pkkv, both wave-0
            # inputs, small consts); the HBM share works out so everything
            # lands by ~13us. Later input waves are issued with a one-column
            # dst OVERLAP into the previous wave: the WAW dependency (plus
            # WAR on the previous wave's readers) is a real dependency the
            # scheduler enforces, so waves 1-3 cannot steal stage-0
            # bandwidth. The overlap column rewrites identical data.
            nc.sync.dma_start(pkq[:], pkq_d[:])
            nc.sync.dma_start(sqt_sb[:, :, 0:512], sqt_d[:, :, 0:512])
            nc.gpsimd.dma_start(pkkv[:], pkkv_d[:])
            nc.gpsimd.dma_start(skvt_sb[:, :, 0:512], skvt_d[:, :, 0:512])
            nc.scalar.dma_start(pk32[:], pk32_d[:])
            for r in (0, 64):
                nc.scalar.dma_start(cost[r : r + 32, :], cost_d[:])
            for r in (0, 32, 64, 96):
                nc.scalar.dma_start(sint[r : r + 32, :], sint_d[:])
            nc.scalar.dma_start(mtile[:], mtile_d[:])
            nc.scalar.dma_start(e64[:], e64_d[:])
            for w in range(1, NS):
                hs = slice(w * 512 - 1, (w + 1) * 512)
                nc.sync.dma_start(sqt_sb[:, :, hs], sqt_d[:, :, hs])
                nc.sync.dma_start(skvt_sb[:, :, hs], skvt_d[:, :, hs])

            # ---- persistent activations ----
            qT = [bigpool.tile([128, N], BF16, tag=f"qT{p}", name=f"qT{p}") for p in range(2)]
            kT = [bigpool.tile([128, N], BF16, tag=f"kT{p}", name=f"kT{p}") for p in range(2)]
            # rotate_every_two copies: head (pr,hh) rot rows at 64*pr+32*hh
            q2all = bigpool.tile([128, N], BF16, tag="q2all", name="q2all")
            k2all = bigpool.tile([128, N], BF16, tag="k2all", name="k2all")
            # vgAll[:, kb, pr, hh, 0:64] = v of head 2pr+hh for key block kb,
            # [:, kb, pr, hh, 64] = ones (denominator column)
            vgAll = bigpool.tile([128, NB, 2, 2, VS], BF16, tag="vg", name="vg")
            oTs = [
                [bigpool.tile([128, 512], BF16, tag=f"oTs{p}_{g}", name=f"oTs{p}_{g}") for g in range(NG)]
                for p in range(2)
            ]
            dsum = bigpool.tile([1, 16, 512], F32, tag="dsum", name="dsum")
            denoms = bigpool.tile([16, 512], F32, tag="denoms", name="denoms")
            rec = bigpool.tile([16, 512], F32, tag="rec", name="rec")
            recb = bigpool.tile([64, 512], BF16, tag="recb", name="recb")

            # hoisted memsets (head of DVE queue); denoms=1.0 so the padded
            # reciprocal rows (alignment rule) stay finite (0*NaN poisons psum)
            nc.vector.memset(vgAll[:], 1.0)
            nc.vector.memset(recb[:], 0.0)
            nc.vector.memset(denoms[:], 1.0)

            def rope_block(dst, dst2, pr, hh, c0, cw):
                """out = q*cos + q2*sin on dst[64*hh:64*hh+32, c0:c0+cw]."""
                cs = slice(c0, c0 + cw)
                r = 64 * hh
                r2 = 64 * pr + 32 * hh
                t1 = smallpool.tile([32, cw], BF16, tag="ropet1", name="ropet1", bufs=2)
                t2 = smallpool.tile([32, cw], BF16, tag="ropet2", name="ropet2", bufs=2)
                v = nc.vector
                v.tensor_mul(t2[:, :], dst2[r2 : r2 + 32, cs], sint[r2 : r2 + 32, cs])
                v.tensor_mul(t1[:, :], dst[r : r + 32, cs], cost[r : r + 32, cs])
                v.tensor_add(dst[r : r + 32, cs], t1[:, :], t2[:, :])

            # ============ phases 1-3: projections + attention, interleaved ============
            with (
                tc.tile_pool(name="pjps", bufs=2, space=bass.MemorySpace.PSUM) as pj,
                tc.tile_pool(name="stps", bufs=1, space=bass.MemorySpace.PSUM) as stq,
                tc.tile_pool(name="otps", bufs=1, space=bass.MemorySpace.PSUM) as otq,
            ):
                def proj_piece_list(s, pi):
                    """Projection unit for sub s split into 3 spool pieces:
                    4 accum MMs, 4 accum MMs, bias evacuation."""
                    cs = slice(s * 512, (s + 1) * 512)
                    projs = [
                        (OQ[0], pk32[:, 0:1], qT[0], sqt_sb),
                        (OQ[1], pk32[:, 1:2], qT[1], sqt_sb),
                        (OK[0], pk32[:, 2:3], kT[0], skvt_sb),
                        (OK[1], pk32[:, 3:4], kT[1], skvt_sb),
                    ]
                    woff, bsb, dst, src_sb = projs[pi]
                    wsb = pkq if pi < 2 else pkkv
                    cell = {}

                    def mk_mm(c0):
                        def piece():
                            if c0 == 0:
                                cell["ps"] = pj.tile([128, 512], F32, tag="pj", name="pj")
                            for c in range(c0, c0 + 4):
                                nc.tensor.matmul(
                                    cell["ps"][:],
                                    wsb[:, woff + c * 128 : woff + (c + 1) * 128],
                                    src_sb[:, c, cs],
                                    start=(c == 0), stop=(c == 7),
                                )
                        return piece

                    def evac():
                        nc.scalar.activation(dst[:, cs], cell["ps"][:], AF.Identity, bias=bsb)

                    return [mk_mm(0), mk_mm(4), evac]

                def rot_piece(s, p, dst2, src):
                    """dst2[64p:64p+64, cs] = rotate_every_two of src[p] via a
                    +/-1 permutation matmul (incl. rotated bias)."""
                    def piece():
                        cs = slice(s * 512, (s + 1) * 512)
                        ps = pj.tile([128, 512], F32, tag="pj", name="pj")
                        nc.tensor.matmul(
                            ps[0:64, :], pkq[:, ORM : ORM + 64], src[p][:, cs],
                            start=True, stop=True,
                        )
                        nc.scalar.activation(
                            dst2[64 * p : 64 * p + 64, cs], ps[0:64, :], AF.Copy
                        )
                    return piece

                def rope_piece(dstsel, s, p, hh):
                    def piece():
                        dst, dst2 = (qT[p], q2all) if dstsel == 0 else (kT[p], k2all)
                        rope_block(dst, dst2, p, hh, s * 512, 512)
                    return piece

                def vt_piece_list(s, half):
                    """vT unit half: 2 pieces of 8 N=256 MMs + evacuations."""
                    cell = {}

                    def mk(kl):
                        def piece():
                            if kl == 0:
                                cell["pv"] = pj.tile([128, 512], F32, tag="pj", name="pj")
                            pv = cell["pv"]
                            kb = 4 * s + 2 * half + kl
                            ks = slice(kb * 128, (kb + 1) * 128)
                            for c in range(8):
                                nc.tensor.matmul(
                                    pv[:, kl * 256 : (kl + 1) * 256],
                                    skvt_sb[:, c, ks],
                                    pkkv[:, OWV + c * 256 : OWV + (c + 1) * 256],
                                    start=(c == 0 and kl == 0),
                                    stop=(c == 7 and kl == 1),
                                )
                            if kl == 1:
                                for kl2 in range(2):
                                    kb2 = 4 * s + 2 * half + kl2
                                    if kl2 == 0:
                                        nc.scalar.activation(
                                            vgAll[:, kb2, :, :, 0:64],
                                            pv[:, 0:256], AF.Copy,
                                        )
                                    else:
                                        nc.vector.tensor_copy(
                                            vgAll[:, kb2, :, :, 0:64], pv[:, 256:512]
                                        )
                        return piece

                    return [mk(0), mk(1)]

                def q0_path(s):
                    """q-pair0 chain for sub s (enables A(0,s) part 1)."""
                    return (proj_piece_list(s, 0) + [rot_piece(s, 0, q2all, qT)]
                            + [rope_piece(0, s, 0, hh) for hh in range(2)])

                def k0vt_path(s):
                    """k-pair0 + vT chain (enables A(0,s) diagonal part)."""
                    return (proj_piece_list(s, 2) + [rot_piece(s, 0, k2all, kT)]
                            + [rope_piece(1, s, 0, hh) for hh in range(2)]
                            + vt_piece_list(s, 0) + vt_piece_list(s, 1))

                def pair1_path(s):
                    """q/k pair-1 chain (enables A(1,s))."""
                    return (proj_piece_list(s, 1) + [rot_piece(s, 1, q2all, qT)]
                            + [rope_piece(0, s, 1, hh) for hh in range(2)]
                            + proj_piece_list(s, 3) + [rot_piece(s, 1, k2all, kT)]
                            + [rope_piece(1, s, 1, hh) for hh in range(2)])

                def norm_piece(g, p):
                    def piece():
                        bc = pj.tile([128, 512], F32, tag="pj", name="pj")
                        nc.tensor.matmul(
                            bc[:], e64[:, p * 4 + g, :], recb[:], start=True, stop=True,
                            tile_position=(0, 0),
                        )
                        nc.vector.tensor_mul(oTs[p][g][:], oTs[p][g][:], bc[:])
                    return piece

                def outproj_pieces(g, pairs=(0, 1)):
                    """normalize (optionally) + out-projection for group g as
                    per-(qb,nh) spool pieces; DMA issued with the second nh."""
                    ps = [norm_piece(g, p) for p in pairs]
                    for qb in range(4 * g, 4 * g + 4):
                        cell = {}
                        for nh in range(2):
                            def piece(qb=qb, nh=nh, cell=cell):
                                if nh == 0:
                                    cell["ob"] = outsb_pool.tile([128, DOUT], BF16, tag="ob", name="ob")
                                off = (qb % 4) * 128
                                po = pj.tile([128, 512], F32, tag="pj", name="pj")
                                for p in range(2):
                                    nc.tensor.matmul(
                                        po[:],
                                        oTs[p][g][:, off : off + 128],
                                        wo_sb[p][:, nh * 512 : (nh + 1) * 512],
                                        start=(p == 0), stop=(p == 1),
                                    )
                                half = cell["ob"][:, nh * 512 : (nh + 1) * 512]
                                if (qb + nh) % 2 == 0:
                                    nc.scalar.activation(half, po[:], AF.Copy)
                                else:
                                    nc.vector.tensor_copy(half, po[:])
                                if nh == 1:
                                    nc.sync.dma_start(
                                        out_ext[qb * 128 : (qb + 1) * 128, :], cell["ob"][:]
                                    )
                            ps.append(piece)
                    return ps

                spool = []

                def drain(n):
                    def filler(kb):
                        for _ in range(n):
                            if spool:
                                spool.pop(0)()
                    return filler

                def flush():
                    while spool:
                        spool.pop(0)()

                def attn_begin():
                    return [
                        otq.tile([128, 512], F32, tag=f"oT{hh}", name=f"oT{hh}")
                        for hh in range(2)
                    ]

                def attn_part(pr, g, oT, kbs, filler=None):
                    for kb in kbs:
                        q0 = max(kb, 4 * g)
                        off = (q0 % 4) * 128
                        qs = slice(g * 512 + off, (g + 1) * 512)
                        ks = slice(kb * 128, (kb + 1) * 128)
                        sT = stq.tile([128, 2, 512], F32, tag="sT", name="sT", bufs=2)
                        nc.tensor.matmul(
                            sT[:, 0, off:], kT[pr][0:64, ks], qT[pr][0:64, qs],
                            start=True, stop=True,
                        )
                        nc.tensor.matmul(
                            sT[:, 1, off:], kT[pr][64:128, ks], qT[pr][64:128, qs],
                            start=True, stop=True,
                        )
                        p = ppool.tile([128, 2, 512], BF16, tag="p", name="p")
                        nc.scalar.activation(
                            p[:, :, off:], sT[:, :, off:], AF.Exp,
                            bias=pk32[:, 4 + kb : 5 + kb], scale=0.125,
                        )
                        if q0 == kb:  # diagonal: zero exp's upper triangle on
                            # the (idle) gpsimd engine
                            nc.gpsimd.tensor_mul(
                                p[:, :, off : off + 128], p[:, :, off : off + 128], mtile[:]
                            )
                        st = kb == 0
                        sp = kb == 4 * g + 3
                        for hh in range(2):
                            nc.tensor.matmul(
                                oT[hh][0:65, off:], vgAll[:, kb, pr, hh, 0:65],
                                p[:, hh, off:], start=st, stop=sp,
                            )
                        if filler is not None:
                            filler(kb)
                    if kbs[-1] == 4 * g + 3:
                        # evacuate unnormalized o and denominator sums
                        for hh in range(2):
                            idx = pr * 8 + g * 2 + hh
                            nc.vector.tensor_copy(
                                oTs[pr][g][hh * 64 : hh * 64 + 64, :], oT[hh][0:64, :]
                            )
                            nc.vector.tensor_copy(dsum[0:1, idx, :], oT[hh][64:65, :])

                def recip_rows(row_slices):
                    for rs in row_slices:
                        nc.sync.dma_start(denoms[rs, :], dsum[0:1, rs, :])
                    nc.vector.reciprocal_approx_fast(rec[:], denoms[:])
                    nc.vector.tensor_copy(recb[0:16, :], rec[:])

                # ============ unified schedule ============
                # Every attention group is split at its causal diagonal; part 1
                # of A(0,g) depends only on the q-pair0 chain of sub g, which
                # was spliced into the PREVIOUS round, so the exp stream never
                # sees an en-bloc projection block. All remaining projection /
                # normalize / out-projection work drains through kb-step
                # fillers at a chosen rate (pieces are ~0.5-1.1us of PE each).
                # A(1,0) (shortest group) runs late so outproj(0) can splice
                # into A(1,3); only outproj(3)-pair1 trails the last exp.
                for f in q0_path(0) + k0vt_path(0):
                    f()
                spool += pair1_path(0)
                oT = attn_begin()
                attn_part(0, 0, oT, range(4), drain(3))
                for p in range(2):
                    nc.gpsimd.dma_start(wo_sb[p][:], wo_d[p])
                flush()
                # ---- round 1 ----
                recip_rows([slice(0, 2)])
                for f in q0_path(1):
                    f()
                spool += [norm_piece(0, 0)] + k0vt_path(1) + pair1_path(1)
                oT = attn_begin()
                attn_part(0, 1, oT, range(4), drain(3))
                attn_part(0, 1, oT, range(4, 8), drain(3))
                spool += q0_path(2)
                oT1 = attn_begin()
                attn_part(1, 1, oT1, range(8), drain(2))
                flush()
                # ---- round 2 ----
                recip_rows([slice(2, 4), slice(10, 12)])
                spool += k0vt_path(2) + pair1_path(2)
                oT = attn_begin()
                attn_part(0, 2, oT, range(8), drain(3))
                spool += outproj_pieces(1)
                attn_part(0, 2, oT, range(8, 12), drain(2))
                spool += q0_path(3)
                oT1 = attn_begin()
                attn_part(1, 2, oT1, range(12), drain(2))
                flush()
                # ---- round 3 / endgame ----
                recip_rows([slice(4, 6), slice(12, 14)])
                spool += k0vt_path(3) + pair1_path(3)
                oT = attn_begin()
                attn_part(0, 3, oT, range(12), drain(3))
                spool += outproj_pieces(2)
                attn_part(0, 3, oT, range(12, 16), drain(2))
                recip_rows([slice(6, 8)])
                spool += [norm_piece(3, 0)]
                oT1 = attn_begin()
                attn_part(1, 0, oT1, range(4), drain(2))
                flush()
                recip_rows([slice(8, 10)])
                spool += outproj_pieces(0, pairs=(1,))
                oT2 = attn_begin()
                attn_part(1, 3, oT2, range(16), drain(1))
                flush()
                recip_rows([slice(14, 16)])
                for f in outproj_pieces(3, pairs=(1,)):
                    f()

    nc.compile()
    return nc


def _rot2(cols):
    """rotate_every_two on the column axis of a [*, 64] block: returns the 32
    rotated columns [-c1, c0, -c3, c2, ...]."""
    out = np.zeros_like(cols[:, :ROT])
    out[:, 0::2] = -cols[:, 1:ROT:2]
    out[:, 1::2] = cols[:, 0:ROT:2]
    return out


def _chunked(w):
    """[1024, 128] -> [128, 1024] with chunk-c cols at c*128."""
    return np.ascontiguousarray(w.reshape(8, 128, 128).transpose(1, 0, 2).reshape(128, 1024))


def _prep_host(s_q, s_kv, mask_q, mask_kv, Wq, bq_, Wkv, bkv_, Wo, bo_):
    inv_freq = 1.0 / (10000.0 ** (np.arange(0, ROT, 2, dtype=np.float64) / ROT))
    t = np.arange(N, dtype=np.float64)[None, :] * inv_freq[:, None]  # [16, N]
    cosT = np.repeat(np.cos(t), 2, axis=0).astype(NPBF16)  # [32, N]
    sinT = np.repeat(np.sin(t), 2, axis=0).astype(NPBF16)

    # lower-triangular keep-mask (key row <= query col), applied to exp(p)
    pidx = np.arange(128)
    mt = (pidx[:, None] <= pidx[None, :]).astype(np.float32)
    mtile2 = np.stack([mt, mt], axis=1).astype(NPBF16)  # [128, 2, 128]

    e64 = np.zeros((64, 8, 128), NPBF16)
    for pr in range(2):
        for g in range(NG):
            e64[pr * 8 + g * 2 + 0, pr * 4 + g, 0:64] = 1.0
            e64[pr * 8 + g * 2 + 1, pr * 4 + g, 64:128] = 1.0
    e64 = e64.reshape(64, 1024)

    # rotate_every_two as a +/-1 permutation on qT/kT partition rows:
    # out row 32*hh + j (head-half hh, rot dim j) reads src row 64*hh + (j+1)
    # with sign -1 for even j, and 64*hh + (j-1) with +1 for odd j.
    rmat = np.zeros((128, 128), np.float32)
    for hh in range(2):
        for j in range(0, ROT, 2):
            rmat[64 * hh + j + 1, 32 * hh + j] = -1.0
            rmat[64 * hh + j, 32 * hh + j + 1] = 1.0
    rmat = rmat.astype(NPBF16)

    in_maps = []
    for core in range(8):
        b = core // 4
        h0 = (core % 4) * HPC

        wq = np.zeros((2, 128, 1024), NPBF16)
        wk = np.zeros((2, 128, 1024), NPBF16)
        bqp = np.zeros((128, 2), np.float32)
        bkp = np.zeros((128, 2), np.float32)
        for pr in range(2):
            cols_q, cols_k, bq_c, bk_c = [], [], [], []
            for hh in range(2):
                h = h0 + 2 * pr + hh
                qcols = Wq[:, h * DH : (h + 1) * DH]
                kcols = Wkv[:, h * 2 * DH : h * 2 * DH + DH]
                cols_q.append(qcols)
                bq_c.append(bq_[h * DH : (h + 1) * DH])
                cols_k.append(kcols)
                bk_c.append(bkv_[h * 2 * DH : h * 2 * DH + DH])
            wq[pr] = _chunked(np.concatenate(cols_q, axis=1)).astype(NPBF16)
            wk[pr] = _chunked(np.concatenate(cols_k, axis=1)).astype(NPBF16)
            bqp[:, pr] = np.concatenate(bq_c)
            bkp[:, pr] = np.concatenate(bk_c)

        # wv: [128, chunk(8) x (pr,hh,dim)(256)]
        wv = np.zeros((8, 128, 256), np.float32)
        for pr in range(2):
            for hh in range(2):
                h = h0 + 2 * pr + hh
                vcols = Wkv[:, h * 2 * DH + DH : (h + 1) * 2 * DH]  # [1024, 64]
                wv[:, :, (pr * 2 + hh) * 64 : (pr * 2 + hh + 1) * 64] = vcols.reshape(
                    8, 128, 64
                )
        wv = np.ascontiguousarray(wv.transpose(1, 0, 2).reshape(128, 2048)).astype(NPBF16)

        wo_rows = Wo[h0 * DH : (h0 + HPC) * DH, :]  # [256, 1024]
        bmask = np.tile(
            (INF * (mask_kv[b].astype(np.float32) - 1.0)).reshape(NB, 128).T[:, :],
            (1, 1),
        )  # [128, NB]

        # pkq: wq0|wq1|rmat [128, 2176]; pkkv: wk0|wk1|wv [128, 4096]
        pkq = np.concatenate([wq[0], wq[1], rmat], axis=1)
        pkkv = np.concatenate([wk[0], wk[1], wv], axis=1)
        # pk32: bq|bk|bmask -> [128, 20] f32
        pk32 = np.concatenate(
            [bqp, bkp, np.ascontiguousarray(bmask).astype(np.float32)], axis=1
        )

        # inputs partition-major: [partition, chunk, N]
        sqtp = np.ascontiguousarray(
            s_q[b].T.astype(NPBF16).reshape(8, 128, N).transpose(1, 0, 2)
        )
        skvtp = np.ascontiguousarray(
            s_kv[b].T.astype(NPBF16).reshape(8, 128, N).transpose(1, 0, 2)
        )

        in_maps.append(
            {
                "sqt": sqtp,
                "skvt": skvtp,
                "pkq": np.ascontiguousarray(pkq),
                "pkkv": np.ascontiguousarray(pkkv),
                "pk32": np.ascontiguousarray(pk32.astype(np.float32)),
                "wo": np.ascontiguousarray(wo_rows.reshape(2, 128, DOUT)).astype(NPBF16),
                "cost": cosT,
                "sint": sinT,
                "mtile": mtile2,
                "e64": e64,
            }
        )
    return in_maps


_NC_CACHE = {}


def kernel(s_q, s_kv, mask_q, mask_kv, Wq, bq, Wkv, bkv, Wo, bo, _return_results=False):
    from concourse.bass_utils import run_bass_kernel_spmd

    if "nc" not in _NC_CACHE:
        _NC_CACHE["nc"] = build_nc()
    nc = _NC_CACHE["nc"]

    s_q = np.asarray(s_q, np.float32)
    s_kv = np.asarray(s_kv, np.float32)
    Wq_ = np.asarray(Wq, np.float32)
    Wkv_ = np.asarray(Wkv, np.float32)
    Wo_ = np.asarray(Wo, np.float32)
    bkv_ = np.asarray(bkv, np.float32)
    in_maps = _prep_host(
        s_q, s_kv,
        np.asarray(mask_q, np.float32),
        np.asarray(mask_kv, np.float32),
        Wq_, np.asarray(bq, np.float32), Wkv_, bkv_, Wo_, np.asarray(bo, np.float32),
    )
    trace = bool(int(os.environ.get("KERNEL_TRACE", "0")))
    res = run_bass_kernel_spmd(nc, in_maps, core_ids=list(range(8)), trace=trace)

    # v-bias contribution folded here: softmax weights sum to 1, so each
    # head's o_norm is missing exactly +bv; add bv_full @ Wo once per batch.
    bv_full = bkv_.reshape(H, 2 * DH)[:, DH:].reshape(-1)
    bo_eff = np.asarray(bo, np.float32) + bv_full @ Wo_

    out = np.zeros((B, N, DOUT), np.float32)
    for core in range(8):
        b = core // 4
        out[b] += res.results[core]["out"].astype(np.float32)
    out += bo_eff[None, None, :]
    if _return_results:
        return out, res
    return out



# revision 36
# speedup vs baseline: 1.0568x; 1.0568x over previous
"""Distributed Trainium2 Bass kernel for nn_AttentionLayer_25993142075512.

Sharding: 8 cores = 2 batches x 4 head-groups (4 heads each). Each core
computes its batch's q/k/v projections for its 4 heads, causal attention,
and a partial output projection o @ Wo[head_rows]. Host sums the 4
partials per batch and adds bo (plus the folded v-bias term).

v2.1 design notes:
  - qT[pr]/kT[pr] hold a HEAD PAIR: head 2pr at partitions 0:64, head 2pr+1
    at 64:128 (natural dim order). Score matmuls for the two heads go to PE
    row tiles (0,0)/(64,0) and run CONCURRENTLY (2x row tiling).
  - RoPE via rotate_every_two copies computed on-device: q2 = R^T q where R
    is a host-built +/-1 permutation [128,64] applied to qT/kT directly (one
    N=512 matmul per pair per sub instead of a full 8-chunk projection);
    rope = q*cos + q2*sin: 3 DVE ops per head per chunk, bases 0 mod 32.
  - vT produced directly in [keys, dims] layout (stationary = s_kv^T chunk,
    moving = Wv columns) - no transpose phase. v bias folded into bo on host.
  - oT matmuls full-K (M=65); softmax denominator comes free via a ones
    column appended to the transposed-v stationary.
  - exp for both heads fused in one ACT op [128, 2, w] over two psum banks;
    the causal diagonal mask is applied AFTER exp as a multiplicative
    0/1-triangle on the (otherwise idle) gpsimd engine, keeping the
    DVE queue out of the exp->oT critical chain.
  - Denominators summed into a free-dim-indexed [1,16,512] tile (engine
    partition bases must be 0 mod 32), spread by SBUF->SBUF DMA, one
    reciprocal, broadcast via one-hot E matmuls (64-mode), DVE multiply.
  - Unified schedule: per projection sub, both head pairs' attention
    groups run back to back; each group's normalize + out-projection is
    emitted one group later, overlapping the exp-paced stream. PSUM:
    proj/bc/po 2x2KB + sT 2x4KB + oT 2x2KB = 16KB exactly.
  - Consolidated host-packed weight DMAs; skvt input DMAs issued from the
    (otherwise idle) gpsimd queue to halve Sync descriptor-issue time.

Assumes mask_q == 1 (spec fill=ones); mask_kv handled exactly via exp bias.
"""

import sys, os, types, ctypes, contextlib

sys.path.insert(0, "/opt/trn_rl_repo")

import numpy as np
import ml_dtypes


def _install_axon_hooks():
    so = "/opt/axon/libaxon_pjrt.so"

    def _hook_factory(so_path):
        if not os.path.exists(so_path):
            return None
        lib = ctypes.CDLL(so_path)
        if not hasattr(lib, "axon_start_nrt_profile"):
            return None
        lib.axon_start_nrt_profile.argtypes = [
            ctypes.POINTER(ctypes.c_int64),
            ctypes.c_size_t,
        ]
        lib.axon_start_nrt_profile.restype = ctypes.c_int64
        lib.axon_stop_nrt_profile.argtypes = [ctypes.c_char_p]
        lib.axon_stop_nrt_profile.restype = ctypes.c_int64

        @contextlib.contextmanager
        def _hook(output_dir, device_ids):
            import jax

            jax.devices()
            if device_ids:
                ids = (ctypes.c_int64 * len(device_ids))(*device_ids)
                rc = lib.axon_start_nrt_profile(ids, len(device_ids))
            else:
                rc = lib.axon_start_nrt_profile(None, 0)
            if rc != 0:
                raise RuntimeError(f"axon_start_nrt_profile rc={rc}")
            try:
                yield
            finally:
                n = lib.axon_stop_nrt_profile(str(output_dir).encode())
                if n < 0:
                    raise RuntimeError(f"axon_stop_nrt_profile rc={n}")

        return _hook

    try:
        import antenv

        if "antenv.axon_hooks" not in sys.modules:
            hook = _hook_factory(so)
            mod = types.ModuleType("antenv.axon_hooks")
            mod.get_axon_ntff_profile_hook = lambda: hook
            mod.set_axon_ntff_profile_hook = lambda h: None
            antenv.axon_hooks = mod
            sys.modules["antenv.axon_hooks"] = mod
    except ImportError:
        pass
    from concourse import bass_utils

    bass_utils.upload_artifacts = lambda tmpdir: tmpdir


_install_axon_hooks()

from concourse import bass, bacc, tile, mybir  # noqa: E402

BF16 = mybir.dt.bfloat16
F32 = mybir.dt.float32
NPBF16 = ml_dtypes.bfloat16

B, N, DQ, DKV, H, DH, DOUT = 2, 2048, 1024, 1024, 16, 64, 1024
ROT = DH // 2  # 32
INF = 1.0e6
HPC = 4  # heads per core (2 pairs)
NB = N // 128  # 16 k-blocks
NG = NB // 4  # 4 q-block groups (512 cols each)
NS = 4  # projection subs (512 cols each)
VS = 66  # vg per-(kb,pr,hh) stride (64 v cols + ones col + pad)


def build_nc():
    nc = bacc.Bacc(None, target_bir_lowering=False)

    # inputs host-packed wave-major: [partition, chunk, wave, 512] so one
    # dma_start moves a whole 1MB column-wave (descriptor issue is ~0.6us per
    # dma_start on the issuing engine queue -- consolidation is critical)
    sqt_d = nc.declare_dram_parameter("sqt", [128, 8, N], BF16, isOutput=False)
    skvt_d = nc.declare_dram_parameter("skvt", [128, 8, N], BF16, isOutput=False)
    # pkq: wq0|wq1|rmat|TBLA packed bf16. DVE tensor-tensor ops need BOTH
    # SB inputs at the SAME base partition, so the rope tables are packed at
    # every base the rope blocks read from: TBLA (in pkq) holds cos at rows
    # 0:32 & 64:96 and sin at rows 32:64 & 96:128; TBLB (in pkkv) holds sin
    # at rows 0:32 & 64:96.
    pkq_d = nc.declare_dram_parameter("pkq", [128, 4224], BF16, isOutput=False)
    pkkv_d = nc.declare_dram_parameter("pkkv", [128, 6144], BF16, isOutput=False)
    # pk32: bq(2)|bk(2)|bmask(16) packed as one f32 [128, 20] tensor
    pk32_d = nc.declare_dram_parameter("pk32", [128, 20], F32, isOutput=False)
    wo_d = nc.declare_dram_parameter("wo", [2, 128, DOUT], BF16, isOutput=False)
    mtile_d = nc.declare_dram_parameter("mtile", [128, 2, 128], BF16, isOutput=False)
    e64_d = nc.declare_dram_parameter("e64", [64, 1024], BF16, isOutput=False)
    out_ext = nc.declare_dram_parameter("out", [N, DOUT], BF16, isOutput=True)

    OQ = (0, 1024)      # wq pair offsets in pkq
    ORM = 2048          # rmat offset in pkq
    OTB = 2176          # TBLA offset in pkq
    OK = (0, 1024)      # wk pair offsets in pkkv
    OWV = 2048          # wv offset in pkkv [128, 8*256]
    OTB2 = 4096         # TBLB offset in pkkv

    AF = mybir.ActivationFunctionType
    ALU = mybir.AluOpType

    with tile.TileContext(nc) as tc:
        with (
            tc.tile_pool(name="const", bufs=1) as cpool,
            tc.tile_pool(name="big", bufs=1) as bigpool,
            tc.tile_pool(name="small", bufs=8) as smallpool,
            tc.tile_pool(name="ptile", bufs=4) as ppool,
            tc.tile_pool(name="outsb", bufs=4) as outsb_pool,
        ):
            # ---- SBUF constants ----
            pkq = cpool.tile([128, 4224], BF16, tag="pkq", name="pkq")
            pkkv = cpool.tile([128, 6144], BF16, tag="pkkv", name="pkkv")
            pk32 = cpool.tile([128, 20], F32, tag="pk32", name="pk32")
            wo_sb = [cpool.tile([128, DOUT], BF16, tag=f"wo{p}", name=f"wo{p}") for p in range(2)]
            mtile = cpool.tile([128, 2, 128], BF16, tag="mtile", name="mtile")
            e64 = cpool.tile([64, 8, 128], BF16, tag="e64", name="e64")

            # full-resident transposed inputs as single tiles [128, chunk, N]
            sqt_sb = bigpool.tile([128, 8, N], BF16, tag="sqt", name="sqt")
            skvt_sb = bigpool.tile([128, 8, N], BF16, tag="skvt", name="skvt")

            # Stage-0 DMAs (pkq incl. rope tables, pkkv, both wave-0 inputs,
            # small consts) run concurrently and land by ~13us; HBM shares
            # per-stream, so the const count is kept minimal. Later input
            # waves use a one-column FORWARD overlap (wave w rewrites the
            # first column of wave w+1 with identical data): the WAW chains
            # wave w+1 behind wave w without coupling to any compute reader,
            # so in-flight streams stay few and early waves keep full BW.
            nc.sync.dma_start(pkq[:], pkq_d[:])
            nc.sync.dma_start(sqt_sb[:, :, 0:513], sqt_d[:, :, 0:513])
            nc.gpsimd.dma_start(pkkv[:], pkkv_d[:])
            nc.gpsimd.dma_start(skvt_sb[:, :, 0:513], skvt_d[:, :, 0:513])
            nc.scalar.dma_start(pk32[:], pk32_d[:])
            nc.scalar.dma_start(mtile[:], mtile_d[:])
            nc.scalar.dma_start(e64[:], e64_d[:])
            for w in range(1, NS):
                hs = slice(w * 512, min((w + 1) * 512 + 1, N))
                nc.sync.dma_start(sqt_sb[:, :, hs], sqt_d[:, :, hs])
                nc.sync.dma_start(skvt_sb[:, :, hs], skvt_d[:, :, hs])

            # ---- persistent activations ----
            qT = [bigpool.tile([128, N], BF16, tag=f"qT{p}", name=f"qT{p}") for p in range(2)]
            kT = [bigpool.tile([128, N], BF16, tag=f"kT{p}", name=f"kT{p}") for p in range(2)]
            # rotate_every_two copies: head (pr,hh) rot rows at 64*pr+32*hh
            q2all = bigpool.tile([128, N], BF16, tag="q2all", name="q2all")
            k2all = bigpool.tile([128, N], BF16, tag="k2all", name="k2all")
            # vgAll[:, kb, pr, hh, 0:64] = v of head 2pr+hh for key block kb,
            # [:, kb, pr, hh, 64] = ones (denominator column)
            vgAll = bigpool.tile([128, NB, 2, 2, VS], BF16, tag="vg", name="vg")
            oTs = [
                [bigpool.tile([128, 512], BF16, tag=f"oTs{p}_{g}", name=f"oTs{p}_{g}") for g in range(NG)]
                for p in range(2)
            ]
            dsum = bigpool.tile([1, 16, 512], F32, tag="dsum", name="dsum")
            denoms = bigpool.tile([16, 512], F32, tag="denoms", name="denoms")
            rec = bigpool.tile([16, 512], F32, tag="rec", name="rec")
            recb = bigpool.tile([64, 512], BF16, tag="recb", name="recb")

            # hoisted memsets (head of DVE queue); denoms=1.0 so the padded
            # reciprocal rows (alignment rule) stay finite (0*NaN poisons psum)
            nc.vector.memset(vgAll[:], 1.0)
            nc.vector.memset(recb[:], 0.0)
            nc.vector.memset(denoms[:], 1.0)

            def rope_block(dst, dst2, pr, hh, c0, cw):
                """out = q*cos + q2*sin on dst[64*hh:64*hh+32, c0:c0+cw]."""
                cs = slice(c0, c0 + cw)
                r = 64 * hh
                r2 = 64 * pr + 32 * hh
                t1 = smallpool.tile([32, cw], BF16, tag="ropet1", name="ropet1", bufs=2)
                t2 = smallpool.tile([32, cw], BF16, tag="ropet2", name="ropet2", bufs=2)
                if r2 in (32, 96):
                    sintab = pkq[r2 : r2 + 32, OTB + c0 : OTB + c0 + cw]
                else:
                    sintab = pkkv[r2 : r2 + 32, OTB2 + c0 : OTB2 + c0 + cw]
                costab = pkq[r : r + 32, OTB + c0 : OTB + c0 + cw]
                v = nc.vector
                v.tensor_mul(t2[:, :], dst2[r2 : r2 + 32, cs], sintab)
                v.tensor_mul(t1[:, :], dst[r : r + 32, cs], costab)
                v.tensor_add(dst[r : r + 32, cs], t1[:, :], t2[:, :])

            # ============ phases 1-3: projections + attention, interleaved ============
            with (
                tc.tile_pool(name="pjps", bufs=2, space=bass.MemorySpace.PSUM) as pj,
                tc.tile_pool(name="stps", bufs=1, space=bass.MemorySpace.PSUM) as stq,
                tc.tile_pool(name="otps", bufs=1, space=bass.MemorySpace.PSUM) as otq,
            ):
                def proj_piece_list(s, pi):
                    """Projection unit for sub s split into 3 spool pieces:
                    4 accum MMs, 4 accum MMs, bias evacuation."""
                    cs = slice(s * 512, (s + 1) * 512)
                    projs = [
                        (OQ[0], pk32[:, 0:1], qT[0], sqt_sb),
                        (OQ[1], pk32[:, 1:2], qT[1], sqt_sb),
                        (OK[0], pk32[:, 2:3], kT[0], skvt_sb),
                        (OK[1], pk32[:, 3:4], kT[1], skvt_sb),
                    ]
                    woff, bsb, dst, src_sb = projs[pi]
                    wsb = pkq if pi < 2 else pkkv
                    cell = {}

                    def mk_mm(c0):
                        def piece():
                            if c0 == 0:
                                cell["ps"] = pj.tile([128, 512], F32, tag="pj", name="pj")
                            for c in range(c0, c0 + 4):
                                nc.tensor.matmul(
                                    cell["ps"][:],
                                    wsb[:, woff + c * 128 : woff + (c + 1) * 128],
                                    src_sb[:, c, cs],
                                    start=(c == 0), stop=(c == 7),
                                )
                        return piece

                    def evac():
                        nc.scalar.activation(dst[:, cs], cell["ps"][:], AF.Identity, bias=bsb)

                    return [mk_mm(0), mk_mm(4), evac]

                def rot_piece(s, p, dst2, src):
                    """dst2[64p:64p+64, cs] = rotate_every_two of src[p] via a
                    +/-1 permutation matmul (incl. rotated bias)."""
                    def piece():
                        cs = slice(s * 512, (s + 1) * 512)
                        ps = pj.tile([128, 512], F32, tag="pj", name="pj")
                        nc.tensor.matmul(
                            ps[0:64, :], pkq[:, ORM : ORM + 64], src[p][:, cs],
                            start=True, stop=True,
                        )
                        nc.scalar.activation(
                            dst2[64 * p : 64 * p + 64, cs], ps[0:64, :], AF.Copy
                        )
                    return piece

                def rope_piece(dstsel, s, p, hh):
                    def piece():
                        dst, dst2 = (qT[p], q2all) if dstsel == 0 else (kT[p], k2all)
                        rope_block(dst, dst2, p, hh, s * 512, 512)
                    return piece

                def vt_piece_list(s, half):
                    """vT unit half: 2 pieces of 8 N=256 MMs + evacuations."""
                    cell = {}

                    def mk(kl):
                        def piece():
                            if kl == 0:
                                cell["pv"] = pj.tile([128, 512], F32, tag="pj", name="pj")
                            pv = cell["pv"]
                            kb = 4 * s + 2 * half + kl
                            ks = slice(kb * 128, (kb + 1) * 128)
                            for c in range(8):
                                nc.tensor.matmul(
                                    pv[:, kl * 256 : (kl + 1) * 256],
                                    skvt_sb[:, c, ks],
                                    pkkv[:, OWV + c * 256 : OWV + (c + 1) * 256],
                                    start=(c == 0 and kl == 0),
                                    stop=(c == 7 and kl == 1),
                                )
                            if kl == 1:
                                for kl2 in range(2):
                                    kb2 = 4 * s + 2 * half + kl2
                                    if kl2 == 0:
                                        nc.scalar.activation(
                                            vgAll[:, kb2, :, :, 0:64],
                                            pv[:, 0:256], AF.Copy,
                                        )
                                    else:
                                        nc.vector.tensor_copy(
                                            vgAll[:, kb2, :, :, 0:64], pv[:, 256:512]
                                        )
                        return piece

                    return [mk(0), mk(1)]

                def q0_path(s):
                    """q-pair0 chain for sub s (enables A(0,s) part 1)."""
                    return (proj_piece_list(s, 0) + [rot_piece(s, 0, q2all, qT)]
                            + [rope_piece(0, s, 0, hh) for hh in range(2)])

                def k0vt_path(s):
                    """k-pair0 + vT chain (enables A(0,s) diagonal part)."""
                    return (proj_piece_list(s, 2) + [rot_piece(s, 0, k2all, kT)]
                            + [rope_piece(1, s, 0, hh) for hh in range(2)]
                            + vt_piece_list(s, 0) + vt_piece_list(s, 1))

                def pair1_path(s):
                    """q/k pair-1 chain (enables A(1,s))."""
                    return (proj_piece_list(s, 1) + [rot_piece(s, 1, q2all, qT)]
                            + [rope_piece(0, s, 1, hh) for hh in range(2)]
                            + proj_piece_list(s, 3) + [rot_piece(s, 1, k2all, kT)]
                            + [rope_piece(1, s, 1, hh) for hh in range(2)])

                def norm_piece(g, p):
                    def piece():
                        bc = pj.tile([128, 512], F32, tag="pj", name="pj")
                        nc.tensor.matmul(
                            bc[:], e64[:, p * 4 + g, :], recb[:], start=True, stop=True,
                            tile_position=(0, 0),
                        )
                        nc.vector.tensor_mul(oTs[p][g][:], oTs[p][g][:], bc[:])
                    return piece

                def outproj_pieces(g, pairs=(0, 1)):
                    """normalize (optionally) + out-projection for group g as
                    per-(qb,nh) spool pieces; DMA issued with the second nh."""
                    ps = [norm_piece(g, p) for p in pairs]
                    for qb in range(4 * g, 4 * g + 4):
                        cell = {}
                        for nh in range(2):
                            def piece(qb=qb, nh=nh, cell=cell):
                                if nh == 0:
                                    cell["ob"] = outsb_pool.tile([128, DOUT], BF16, tag="ob", name="ob")
                                off = (qb % 4) * 128
                                po = pj.tile([128, 512], F32, tag="pj", name="pj")
                                for p in range(2):
                                    nc.tensor.matmul(
                                        po[:],
                                        oTs[p][g][:, off : off + 128],
                                        wo_sb[p][:, nh * 512 : (nh + 1) * 512],
                                        start=(p == 0), stop=(p == 1),
                                    )
                                half = cell["ob"][:, nh * 512 : (nh + 1) * 512]
                                if (qb + nh) % 2 == 0:
                                    nc.scalar.activation(half, po[:], AF.Copy)
                                else:
                                    nc.vector.tensor_copy(half, po[:])
                                if nh == 1:
                                    nc.sync.dma_start(
                                        out_ext[qb * 128 : (qb + 1) * 128, :], cell["ob"][:]
                                    )
                            ps.append(piece)
                    return ps

                spool = []

                def drain(n):
                    def filler(kb):
                        for _ in range(n):
                            if spool:
                                spool.pop(0)()
                    return filler

                def flush():
                    while spool:
                        spool.pop(0)()

                def attn_begin():
                    return [
                        otq.tile([128, 512], F32, tag=f"oT{hh}", name=f"oT{hh}")
                        for hh in range(2)
                    ]

                def attn_part(pr, g, oT, kbs, filler=None):
                    for kb in kbs:
                        q0 = max(kb, 4 * g)
                        off = (q0 % 4) * 128
                        qs = slice(g * 512 + off, (g + 1) * 512)
                        ks = slice(kb * 128, (kb + 1) * 128)
                        sT = stq.tile([128, 2, 512], F32, tag="sT", name="sT", bufs=2)
                        nc.tensor.matmul(
                            sT[:, 0, off:], kT[pr][0:64, ks], qT[pr][0:64, qs],
                            start=True, stop=True,
                        )
                        nc.tensor.matmul(
                            sT[:, 1, off:], kT[pr][64:128, ks], qT[pr][64:128, qs],
                            start=True, stop=True,
                        )
                        p = ppool.tile([128, 2, 512], BF16, tag="p", name="p")
                        nc.scalar.activation(
                            p[:, :, off:], sT[:, :, off:], AF.Exp,
                            bias=pk32[:, 4 + kb : 5 + kb], scale=0.125,
                        )
                        if q0 == kb:  # diagonal: zero exp's upper triangle on
                            # the (idle) gpsimd engine
                            nc.gpsimd.tensor_mul(
                                p[:, :, off : off + 128], p[:, :, off : off + 128], mtile[:]
                            )
                        st = kb == 0
                        sp = kb == 4 * g + 3
                        for hh in range(2):
                            nc.tensor.matmul(
                                oT[hh][0:65, off:], vgAll[:, kb, pr, hh, 0:65],
                                p[:, hh, off:], start=st, stop=sp,
                            )
                        if filler is not None:
                            filler(kb)
                    if kbs[-1] == 4 * g + 3:
                        # evacuate unnormalized o and denominator sums
                        for hh in range(2):
                            idx = pr * 8 + g * 2 + hh
                            nc.vector.tensor_copy(
                                oTs[pr][g][hh * 64 : hh * 64 + 64, :], oT[hh][0:64, :]
                            )
                            nc.vector.tensor_copy(dsum[0:1, idx, :], oT[hh][64:65, :])

                def recip_rows(row_slices):
                    for rs in row_slices:
                        nc.sync.dma_start(denoms[rs, :], dsum[0:1, rs, :])
                    nc.vector.reciprocal_approx_fast(rec[:], denoms[:])
                    nc.vector.tensor_copy(recb[0:16, :], rec[:])

                # ============ unified schedule ============
                # Every attention group is split at its causal diagonal; part 1
                # of A(0,g) depends only on the q-pair0 chain of sub g, which
                # was spliced into the PREVIOUS round, so the exp stream never
                # sees an en-bloc projection block. All remaining projection /
                # normalize / out-projection work drains through kb-step
                # fillers at a chosen rate (pieces are ~0.5-1.1us of PE each).
                # A(1,0) (shortest group) runs late so outproj(0) can splice
                # into A(1,3); only outproj(3)-pair1 trails the last exp.
                for f in q0_path(0) + k0vt_path(0):
                    f()
                spool += pair1_path(0)
                oT = attn_begin()
                attn_part(0, 0, oT, range(4), drain(3))
                for p in range(2):
                    nc.gpsimd.dma_start(wo_sb[p][:], wo_d[p])
                flush()
                # ---- round 1 ----
                recip_rows([slice(0, 2)])
                for f in q0_path(1):
                    f()
                spool += [norm_piece(0, 0)] + k0vt_path(1) + pair1_path(1)
                oT = attn_begin()
                attn_part(0, 1, oT, range(4), drain(3))
                attn_part(0, 1, oT, range(4, 8), drain(3))
                spool += q0_path(2)
                oT1 = attn_begin()
                attn_part(1, 1, oT1, range(8), drain(2))
                flush()
                # ---- round 2 ----
                recip_rows([slice(2, 4), slice(10, 12)])
                spool += k0vt_path(2) + pair1_path(2)
                oT = attn_begin()
                attn_part(0, 2, oT, range(8), drain(3))
                spool += outproj_pieces(1)
                attn_part(0, 2, oT, range(8, 12), drain(2))
                spool += q0_path(3)
                oT1 = attn_begin()
                attn_part(1, 2, oT1, range(12), drain(2))
                flush()
                # ---- round 3 / endgame ----
                recip_rows([slice(4, 6), slice(12, 14)])
                spool += k0vt_path(3) + pair1_path(3)
                oT = attn_begin()
                attn_part(0, 3, oT, range(12), drain(3))
                spool += outproj_pieces(2)
                attn_part(0, 3, oT, range(12, 16), drain(2))
                recip_rows([slice(6, 8)])
                spool += [norm_piece(3, 0)]
                oT1 = attn_begin()
                attn_part(1, 0, oT1, range(4), drain(2))
                flush()
                recip_rows([slice(8, 10)])
                spool += outproj_pieces(0, pairs=(1,))
                oT2 = attn_begin()
                attn_part(1, 3, oT2, range(16), drain(1))
                flush()
                recip_rows([slice(14, 16)])
                for f in outproj_pieces(3, pairs=(1,)):
                    f()

    nc.compile()
    return nc


def _rot2(cols):
    """rotate_every_two on the column axis of a [*, 64] block: returns the 32
    rotated columns [-c1, c0, -c3, c2, ...]."""
    out = np.zeros_like(cols[:, :ROT])
    out[:, 0::2] = -cols[:, 1:ROT:2]
    out[:, 1::2] = cols[:, 0:ROT:2]
    return out


def _chunked(w):
    """[1024, 128] -> [128, 1024] with chunk-c cols at c*128."""
    return np.ascontiguousarray(w.reshape(8, 128, 128).transpose(1, 0, 2).reshape(128, 1024))


def _prep_host(s_q, s_kv, mask_q, mask_kv, Wq, bq_, Wkv, bkv_, Wo, bo_):
    inv_freq = 1.0 / (10000.0 ** (np.arange(0, ROT, 2, dtype=np.float64) / ROT))
    t = np.arange(N, dtype=np.float64)[None, :] * inv_freq[:, None]  # [16, N]
    cosT = np.repeat(np.cos(t), 2, axis=0).astype(NPBF16)  # [32, N]
    sinT = np.repeat(np.sin(t), 2, axis=0).astype(NPBF16)
    tbla = np.zeros((128, N), NPBF16)
    tbla[0:32] = cosT
    tbla[32:64] = sinT
    tbla[64:96] = cosT
    tbla[96:128] = sinT
    tblb = np.zeros((128, N), NPBF16)
    tblb[0:32] = sinT
    tblb[64:96] = sinT

    # lower-triangular keep-mask (key row <= query col), applied to exp(p)
    pidx = np.arange(128)
    mt = (pidx[:, None] <= pidx[None, :]).astype(np.float32)
    mtile2 = np.stack([mt, mt], axis=1).astype(NPBF16)  # [128, 2, 128]

    e64 = np.zeros((64, 8, 128), NPBF16)
    for pr in range(2):
        for g in range(NG):
            e64[pr * 8 + g * 2 + 0, pr * 4 + g, 0:64] = 1.0
            e64[pr * 8 + g * 2 + 1, pr * 4 + g, 64:128] = 1.0
    e64 = e64.reshape(64, 1024)

    # rotate_every_two as a +/-1 permutation on qT/kT partition rows:
    # out row 32*hh + j (head-half hh, rot dim j) reads src row 64*hh + (j+1)
    # with sign -1 for even j, and 64*hh + (j-1) with +1 for odd j.
    rmat = np.zeros((128, 128), np.float32)
    for hh in range(2):
        for j in range(0, ROT, 2):
            rmat[64 * hh + j + 1, 32 * hh + j] = -1.0
            rmat[64 * hh + j, 32 * hh + j + 1] = 1.0
    rmat = rmat.astype(NPBF16)

    in_maps = []
    for core in range(8):
        b = core // 4
        h0 = (core % 4) * HPC

        wq = np.zeros((2, 128, 1024), NPBF16)
        wk = np.zeros((2, 128, 1024), NPBF16)
        bqp = np.zeros((128, 2), np.float32)
        bkp = np.zeros((128, 2), np.float32)
        for pr in range(2):
            cols_q, cols_k, bq_c, bk_c = [], [], [], []
            for hh in range(2):
                h = h0 + 2 * pr + hh
                qcols = Wq[:, h * DH : (h + 1) * DH]
                kcols = Wkv[:, h * 2 * DH : h * 2 * DH + DH]
                cols_q.append(qcols)
                bq_c.append(bq_[h * DH : (h + 1) * DH])
                cols_k.append(kcols)
                bk_c.append(bkv_[h * 2 * DH : h * 2 * DH + DH])
            wq[pr] = _chunked(np.concatenate(cols_q, axis=1)).astype(NPBF16)
            wk[pr] = _chunked(np.concatenate(cols_k, axis=1)).astype(NPBF16)
            bqp[:, pr] = np.concatenate(bq_c)
            bkp[:, pr] = np.concatenate(bk_c)

        # wv: [128, chunk(8) x (pr,hh,dim)(256)]
        wv = np.zeros((8, 128, 256), np.float32)
        for pr in range(2):
            for hh in range(2):
                h = h0 + 2 * pr + hh
                vcols = Wkv[:, h * 2 * DH + DH : (h + 1) * 2 * DH]  # [1024, 64]
                wv[:, :, (pr * 2 + hh) * 64 : (pr * 2 + hh + 1) * 64] = vcols.reshape(
                    8, 128, 64
                )
        wv = np.ascontiguousarray(wv.transpose(1, 0, 2).reshape(128, 2048)).astype(NPBF16)

        wo_rows = Wo[h0 * DH : (h0 + HPC) * DH, :]  # [256, 1024]
        bmask = np.tile(
            (INF * (mask_kv[b].astype(np.float32) - 1.0)).reshape(NB, 128).T[:, :],
            (1, 1),
        )  # [128, NB]

        # pkq: wq0|wq1|rmat|TBLA [128, 4224]; pkkv: wk0|wk1|wv|TBLB
        pkq = np.concatenate([wq[0], wq[1], rmat, tbla], axis=1)
        pkkv = np.concatenate([wk[0], wk[1], wv, tblb], axis=1)
        # pk32: bq|bk|bmask -> [128, 20] f32
        pk32 = np.concatenate(
            [bqp, bkp, np.ascontiguousarray(bmask).astype(np.float32)], axis=1
        )

        # inputs partition-major: [partition, chunk, N]
        sqtp = np.ascontiguousarray(
            s_q[b].T.astype(NPBF16).reshape(8, 128, N).transpose(1, 0, 2)
        )
        skvtp = np.ascontiguousarray(
            s_kv[b].T.astype(NPBF16).reshape(8, 128, N).transpose(1, 0, 2)
        )

        in_maps.append(
            {
                "sqt": sqtp,
                "skvt": skvtp,
                "pkq": np.ascontiguousarray(pkq),
                "pkkv": np.ascontiguousarray(pkkv),
                "pk32": np.ascontiguousarray(pk32.astype(np.float32)),
                "wo": np.ascontiguousarray(wo_rows.reshape(2, 128, DOUT)).astype(NPBF16),
                "mtile": mtile2,
                "e64": e64,
            }
        )
    return in_maps


_NC_CACHE = {}


def kernel(s_q, s_kv, mask_q, mask_kv, Wq, bq, Wkv, bkv, Wo, bo, _return_results=False):
    from concourse.bass_utils import run_bass_kernel_spmd

    if "nc" not in _NC_CACHE:
        _NC_CACHE["nc"] = build_nc()
    nc = _NC_CACHE["nc"]

    s_q = np.asarray(s_q, np.float32)
    s_kv = np.asarray(s_kv, np.float32)
    Wq_ = np.asarray(Wq, np.float32)
    Wkv_ = np.asarray(Wkv, np.float32)
    Wo_ = np.asarray(Wo, np.float32)
    bkv_ = np.asarray(bkv, np.float32)
    in_maps = _prep_host(
        s_q, s_kv,
        np.asarray(mask_q, np.float32),
        np.asarray(mask_kv, np.float32),
        Wq_, np.asarray(bq, np.float32), Wkv_, bkv_, Wo_, np.asarray(bo, np.float32),
    )
    trace = bool(int(os.environ.get("KERNEL_TRACE", "0")))
    res = run_bass_kernel_spmd(nc, in_maps, core_ids=list(range(8)), trace=trace)

    # v-bias contribution folded here: softmax weights sum to 1, so each
    # head's o_norm is missing exactly +bv; add bv_full @ Wo once per batch.
    bv_full = bkv_.reshape(H, 2 * DH)[:, DH:].reshape(-1)
    bo_eff = np.asarray(bo, np.float32) + bv_full @ Wo_

    out = np.zeros((B, N, DOUT), np.float32)
    for core in range(8):
        b = core // 4
        out[b] += res.results[core]["out"].astype(np.float32)
    out += bo_eff[None, None, :]
    if _return_results:
        return out, res
    return out

